# revision 2
# baseline (speedup 1.0000x reference)
"""Multi-camera cross-attention (BEVFormer-style) Trainium2 kernel.

Strategy (8 NeuronCores):
  - batch b=0 -> cores 0-3, b=1 -> cores 4-7. Within a group of 4 cores the 6
    cameras are split 1.5/core: each core owns one full camera (900 queries)
    plus half of another camera (450 queries).
  - Host precomputes (numpy f32, replicating the reference math exactly):
    projection -> per (cam, level, query) a 2x2-patch base index and 4 slot
    weights (bilinear weights x validity / 4). Queries whose weights are all
    zero for a camera (~52%) are compacted away.
  - Features are repacked on host to fp16 "patch rows": row (lvl_off+y*W+x) =
    the 4 pixels [(y,x),(y,x+1),(y+1,x),(y+1,x+1)] x 256 ch = 2 KB.
  - Device: dma_gather patch rows (one index per live query x level), FMA the
    4 slots with per-partition scalar weights (queries on partitions),
    dma_scatter_add to un-compact into a zeroed DRAM staging buffer,
    DMA-transpose read back (channels on partitions), Wv matmul, per-group
    AllReduce-max over 4 cores (= max over that batch's 6 cams), then the
    fused residual + Wo projection in c-on-partition layout.
"""
import sys
sys.path.insert(0, '/opt/trn_rl_repo')
import numpy as np

MIN_R, MAX_R = -51.2, 51.2
ORIG_W, ORIG_H = 800.0, 448.0
LEVELS = [(112, 200), (56, 100), (28, 50), (14, 25)]
LVL_OFF = [0, 22400, 28000, 29400]
NPIX = 29750
N_CORES = 8
QF, QH = 900, 450
NEG = -1.0e30


# ---------------------------------------------------------------- host math
def _project(reference_points, key_padding_mask, cam_intrinsics, cam_extrinsics):
    """idx [B,N,4,Q] int32 (patch base w/ level offset) ; w [B,N,4,4,Q] f32."""
    B, Q, _ = reference_points.shape
    f32 = np.float32
    ref = reference_points.astype(f32) * f32(MAX_R - MIN_R) + f32(MIN_R)
    ref = np.where(key_padding_mask[..., None], f32(-1000.0), ref)
    ref_hom = np.concatenate([ref, np.ones((B, Q, 1), f32)], axis=-1)
    inv_ext = np.linalg.inv(cam_extrinsics.astype(np.float32))
    inv_ext = np.nan_to_num(inv_ext, nan=0.0, posinf=1e6, neginf=-1e6).astype(f32)
    pts_cam_hom = np.einsum('bqj,bnij->bnqi', ref_hom, inv_ext).astype(f32)
    depth = np.nan_to_num(pts_cam_hom[..., 2:3], nan=10.0, posinf=100.0,
                          neginf=-100.0).astype(f32)
    invalid = depth[..., 0] < f32(1.5)
    depth_safe = np.maximum(depth, f32(1.5))
    pts_cam = (pts_cam_hom[..., :3] / depth_safe).astype(f32)
    pts_img = np.einsum('bnqj,bnij->bnqi', pts_cam,
                        cam_intrinsics.astype(f32))[..., :2].astype(f32)
    pts_img = np.clip(pts_img, -3000.0, 3000.0).astype(f32)

    idx_all = np.zeros((B, 6, 4, Q), np.int32)
    w_all = np.zeros((B, 6, 4, 4, Q), np.float32)
    for li, (Hf, Wf) in enumerate(LEVELS):
        fx = pts_img[..., 0] * f32(Wf / ORIG_W)
        fy = pts_img[..., 1] * f32(Hf / ORIG_H)
        gx = np.clip(fx / f32(Wf - 1.0) * f32(2.0) - f32(1.0), -10.0, 10.0).astype(f32)
        gy = np.clip(fy / f32(Hf - 1.0) * f32(2.0) - f32(1.0), -10.0, 10.0).astype(f32)
        gx = np.where(invalid, f32(-100.0), gx)
        gy = np.where(invalid, f32(-100.0), gy)
        px = (gx + f32(1.0)) * f32(0.5) * f32(Wf - 1)
        py = (gy + f32(1.0)) * f32(0.5) * f32(Hf - 1)
        x0 = np.floor(px)
        y0 = np.floor(py)
        x0i = x0.astype(np.int32)
        y0i = y0.astype(np.int32)
        wx1 = px - x0
        wy1 = py - y0
        bx = np.clip(x0i, 0, Wf - 2)
        by = np.clip(y0i, 0, Hf - 2)
        idx_all[:, :, li] = LVL_OFF[li] + by * Wf + bx
        for dx, dy, w in ((0, 0, (1 - wx1) * (1 - wy1)), (1, 0, wx1 * (1 - wy1)),
                          (0, 1, (1 - wx1) * wy1), (1, 1, wx1 * wy1)):
            xi = x0i + dx
            yi = y0i + dy
            valid = (xi >= 0) & (xi < Wf) & (yi >= 0) & (yi < Hf)
            pcx = np.clip(xi, 0, Wf - 1)
            pcy = np.clip(yi, 0, Hf - 1)
            slot = (pcx - bx) + 2 * (pcy - by)
            wv = (w * valid).astype(f32) * f32(0.25)
            for s in range(4):
                w_all[:, :, li, s] += np.where(slot == s, wv, 0.0)
    return idx_all, w_all


def _build_patches(feats_list):
    """4x [B,N,C,H,W] f32 -> [B,N,NPIX,1024] fp16 patch rows."""
    B, N, C = feats_list[0].shape[:3]
    out = np.empty((B, N, NPIX, 4 * C), np.float16)
    for li, (Hf, Wf) in enumerate(LEVELS):
        F = feats_list[li].astype(np.float16)
        A = np.ascontiguousarray(F.transpose(0, 1, 3, 4, 2))  # [B,N,H,W,C]
        x1 = np.minimum(np.arange(Wf) + 1, Wf - 1)
        y1 = np.minimum(np.arange(Hf) + 1, Hf - 1)
        Ax = A[:, :, :, x1]
        Ay = A[:, :, y1]
        Axy = Ay[:, :, :, x1]
        P = np.stack([A, Ax, Ay, Axy], axis=4)  # [B,N,H,W,4,C]
        out[:, :, LVL_OFF[li]:LVL_OFF[li] + Hf * Wf] = P.reshape(B, N, Hf * Wf, 4 * C)
    return out


def _pack_idx(idx, npad, pad):
    """[k] ints (k<=npad, npad%16==0) -> [128, npad//16] int16 wrapped, x8."""
    buf = np.full((16, npad // 16), pad, np.int16)
    q = len(idx)
    buf[np.arange(q) % 16, np.arange(q) // 16] = np.asarray(idx, np.int16)
    return np.tile(buf, (8, 1))


def _pack_w(w, J):
    """w [4,4,k] -> [128, 4, 4, J] f32 (pos -> partition pos%128, slot pos//128)."""
    k = w.shape[-1]
    out = np.zeros((128, 4, 4, J), np.float32)
    qi = np.arange(k)
    out[qi % 128, :, :, qi // 128] = w.transpose(2, 0, 1)
    return out


def _wt4(W):
    """[256,256] weight -> lhsT pack [128, 2, 2, 128] fp16: [p,kb,mb,m]."""
    return np.ascontiguousarray(
        W.T.reshape(2, 128, 2, 128).transpose(1, 0, 2, 3)).astype(np.float16)


# ---------------------------------------------------------------- device graph
_GRAPHS = {}


def _graph(JF, JH):
    if (JF, JH) in _GRAPHS:
        return _GRAPHS[(JF, JH)]
    import concourse.bacc as bacc
    import concourse.mybir as mybir
    from concourse.tile import TileContext
    from concourse.tile_rust import add_dep_helper

    f16 = mybir.dt.float16
    f32 = mybir.dt.float32
    i16 = mybir.dt.int16
    i32 = mybir.dt.int32
    ALU = mybir.AluOpType
    ACTF = mybir.ActivationFunctionType
    NF, NH = JF * 128, JH * 128          # padded live-token counts
    MSF, MSH = 1024, 512                 # ms staging rows (>= QF / QH, mult 128)

    nc = bacc.Bacc(None, num_devices=N_CORES, dynamic_dma_scratch_size=49152)
    feats_f = nc.dram_tensor("feats_f", [NPIX, 1024], f16, kind="ExternalInput")
    feats_h = nc.dram_tensor("feats_h", [NPIX, 1024], f16, kind="ExternalInput")
    idx_f = nc.dram_tensor("idx_f", [4, 128, NF // 16], i16, kind="ExternalInput")
    idx_h = nc.dram_tensor("idx_h", [4, 128, NH // 16], i16, kind="ExternalInput")
    sidx_f = nc.dram_tensor("sidx_f", [128, NF // 16], i16, kind="ExternalInput")
    sidx_h = nc.dram_tensor("sidx_h", [128, NH // 16], i16, kind="ExternalInput")
    w_f = nc.dram_tensor("w_f", [128, 4, 4, JF], f32, kind="ExternalInput")
    w_h = nc.dram_tensor("w_h", [128, 4, 4, JH], f32, kind="ExternalInput")
    blend = nc.dram_tensor("blend", [128, 4], f32, kind="ExternalInput")
    queryT = nc.dram_tensor("queryT", [128, 2, QF], f16, kind="ExternalInput")
    wq = nc.dram_tensor("wq", [128, 2, 2, 128], f16, kind="ExternalInput")
    wv = nc.dram_tensor("wv", [128, 2, 2, 128], f16, kind="ExternalInput")
    wo = nc.dram_tensor("wo", [128, 2, 2, 128], f16, kind="ExternalInput")
    bq = nc.dram_tensor("bq", [128, 2], f32, kind="ExternalInput")
    bv = nc.dram_tensor("bv", [128, 2], f32, kind="ExternalInput")
    bo = nc.dram_tensor("bo", [128, 2], f32, kind="ExternalInput")
    out_d = nc.dram_tensor("out", [128, 2, QF], f32, kind="ExternalOutput")
    ms_f = nc.dram_tensor("ms_f", [MSF, 256], f16, kind="Internal")
    ms_h = nc.dram_tensor("ms_h", [MSH, 256], f16, kind="Internal")
    cc_in = nc.dram_tensor("cc_in", [128, 2, QF], f16, kind="Internal")
    cc_out = nc.dram_tensor("cc_out", [128, 2, QF], f16, kind="Internal")

    with TileContext(nc) as tc:
        with (
            tc.tile_pool(name="const", bufs=1) as cp,
            tc.tile_pool(name="g", bufs=3) as gp,
            tc.tile_pool(name="wk", bufs=1) as wk,
            tc.tile_pool(name="pv", bufs=4, space="PSUM") as pvp,
        ):
            idxf_t = cp.tile([128, 4, NF // 16], i16)
            nc.sync.dma_start(out=idxf_t[:], in_=idx_f.rearrange("l p s -> p l s"))
            idxh_t = cp.tile([128, 4, NH // 16], i16)
            nc.sync.dma_start(out=idxh_t[:], in_=idx_h.rearrange("l p s -> p l s"))
            sidxf_t = cp.tile([128, NF // 16], i16)
            nc.sync.dma_start(out=sidxf_t[:], in_=sidx_f[:])
            sidxh_t = cp.tile([128, NH // 16], i16)
            nc.sync.dma_start(out=sidxh_t[:], in_=sidx_h[:])
            wf_t = cp.tile([128, 4, 4, JF], f32)
            nc.sync.dma_start(out=wf_t[:], in_=w_f[:])
            wh_t = cp.tile([128, 4, 4, JH], f32)
            nc.sync.dma_start(out=wh_t[:], in_=w_h[:])
            blend_t = cp.tile([128, 4], f32)
            nc.sync.dma_start(out=blend_t[:], in_=blend[:])
            queryT_t = cp.tile([128, 2, QF], f16)
            nc.sync.dma_start(out=queryT_t[:], in_=queryT[:])
            wq_t = cp.tile([128, 2, 2, 128], f16)
            nc.sync.dma_start(out=wq_t[:], in_=wq[:])
            wv_t = cp.tile([128, 2, 2, 128], f16)
            nc.sync.dma_start(out=wv_t[:], in_=wv[:])
            wo_t = cp.tile([128, 2, 2, 128], f16)
            nc.sync.dma_start(out=wo_t[:], in_=wo[:])
            bq_t = cp.tile([128, 2], f32)
            nc.sync.dma_start(out=bq_t[:], in_=bq[:])
            bv_t = cp.tile([128, 2], f32)
            nc.sync.dma_start(out=bv_t[:], in_=bv[:])
            bo_t = cp.tile([128, 2], f32)
            nc.sync.dma_start(out=bo_t[:], in_=bo[:])
            zero_t = cp.tile([128, 2048], f16)
            nc.vector.memset(zero_t[:], 0.0)


            def matmuls(lhsT_t, rhs_get, nch, chw, psname):
                for mb in range(2):
                    for ch in range(nch):
                        pt = pvp.tile([128, chw], f32, tag="pv",
                                      name=f"{psname}_{mb}_{ch}")
                        for kb in range(2):
                            nc.tensor.matmul(pt[:], lhsT=lhsT_t[:, kb, mb, :],
                                             rhs=rhs_get(kb, ch),
                                             start=(kb == 0), stop=(kb == 1))
                        yield mb, ch, pt

            def gather_fma(feats_dram, idx_t, w_t, J, acc, lvl, tag):
                G = gp.tile([128, J, 1024], f16, tag=f"g_{tag}",
                            name=f"G_{tag}_{lvl}")
                nc.gpsimd.dma_gather(G[:], feats_dram[:, :], idx_t[:, lvl, :],
                                     J * 128, J * 128, 1024, elem_step=1024)
                for j in range(J):
                    for ps in range(4):
                        in0 = G[:, j, ps * 256:(ps + 1) * 256]
                        sc = w_t[:, lvl, ps, j:j + 1]
                        if lvl == 0 and ps == 0:
                            nc.vector.tensor_scalar_mul(acc[:, j, :], in0, sc)
                        else:
                            nc.vector.scalar_tensor_tensor(
                                acc[:, j, :], in0, sc, acc[:, j, :],
                                ALU.mult, ALU.add)

            def slab_tail(acc, sidx_t, J, msd, MS, Q, tag):
                zi = nc.sync.dma_start(
                    out=msd.rearrange("(a p) c -> p a c", p=128),
                    in_=zero_t[:, 0:(MS // 128) * 256])
                si = nc.gpsimd.dma_scatter_add(msd[:, :], acc[:], sidx_t[:],
                                               J * 128, J * 128, 256)
                add_dep_helper(si.ins, zi.ins, reason=f"scatter after zero {tag}")
                msT = wk.tile([128, 2, MS], f16, name=f"msT_{tag}")
                for cb in range(2):
                    ti = nc.sync.dma_start(out=msT[:, cb, :],
                                           in_=msd[:, cb * 128:(cb + 1) * 128],
                                           transpose=True)
                    add_dep_helper(ti.ins, si.ins, reason=f"tread after scatter {tag}")
                v = wk.tile([128, 2, Q], f32, name=f"v_{tag}")
                for mb, ch, pt in matmuls(wv_t,
                                          lambda kb, ch: msT[:, kb, ch * 450:(ch + 1) * 450],
                                          Q // 450, 450, f"pv_{tag}"):
                    nc.scalar.activation(v[:, mb, ch * 450:(ch + 1) * 450], pt[:],
                                         ACTF.Copy)
                return v

            acc_f = wk.tile([128, JF, 256], f16, name="acc_f")
            acc_h = wk.tile([128, JH, 256], f16, name="acc_h")
            for lvl in range(4):
                gather_fma(feats_f, idxf_t, wf_t, JF, acc_f, lvl, "f")
                gather_fma(feats_h, idxh_t, wh_t, JH, acc_h, lvl, "h")
            vf = slab_tail(acc_f, sidxf_t, JF, ms_f, MSF, QF, "f")
            vh = slab_tail(acc_h, sidxh_t, JH, ms_h, MSH, QH, "h")

            bounce = wk.tile([128, 2, QF], f16)
            for half in range(2):
                hb = wk.tile([128, 2, QH], f32, tag=f"hb{half}", name=f"hb{half}")
                nc.vector.tensor_scalar(hb[:], vh[:], blend_t[:, 2 * half:2 * half + 1],
                                        blend_t[:, 2 * half + 1:2 * half + 2],
                                        ALU.mult, ALU.add)
                nc.vector.tensor_tensor(
                    out=bounce[:, :, half * QH:(half + 1) * QH],
                    in0=vf[:, :, half * QH:(half + 1) * QH], in1=hb[:], op=ALU.max)
                nc.sync.dma_start(out=cc_in[:, :, half * QH:(half + 1) * QH],
                                  in_=bounce[:, :, half * QH:(half + 1) * QH])
            nc.gpsimd.collective_compute(
                "AllReduce", ALU.max,
                replica_groups=[[0, 1, 2, 3], [4, 5, 6, 7]],
                ins=[cc_in[:]], outs=[cc_out[:]])
            sT16 = wk.tile([128, 2, QF], f16)
            nc.sync.dma_start(out=sT16[:], in_=cc_out[:])
            sT = wk.tile([128, 2, QF], f32)
            nc.scalar.activation(sT[:], sT16[:], ACTF.Copy)

            qT = wk.tile([128, 2, QF], f32)
            for mb, ch, pt in matmuls(wq_t,
                                      lambda kb, ch: queryT_t[:, kb, ch * 450:(ch + 1) * 450],
                                      2, 450, "pq"):
                nc.vector.tensor_scalar_add(qT[:, mb, ch * 450:(ch + 1) * 450], pt[:],
                                            bq_t[:, mb:mb + 1])
            t1 = wk.tile([128, 2, QF], f32)
            for mb in range(2):
                nc.vector.scalar_tensor_tensor(t1[:, mb, :], sT[:, mb, :],
                                               bv_t[:, mb:mb + 1], qT[:, mb, :],
                                               ALU.add, ALU.add)
            t2 = wk.tile([128, 2, QF], f32)
            nc.scalar.activation(t2[:], t1[:], ACTF.Relu)
            fusedT = wk.tile([128, 2, QF], f16)
            nc.vector.tensor_tensor(out=fusedT[:], in0=t2[:], in1=qT[:], op=ALU.add)
            outT = wk.tile([128, 2, QF], f32)
            for mb, ch, pt in matmuls(wo_t,
                                      lambda kb, ch: fusedT[:, kb, ch * 450:(ch + 1) * 450],
                                      2, 450, "po"):
                nc.vector.tensor_scalar_add(outT[:, mb, ch * 450:(ch + 1) * 450], pt[:],
                                            bo_t[:, mb:mb + 1])
            nc.sync.dma_start(out=out_d[:], in_=outT[:])
    nc.compile()
    _GRAPHS[(JF, JH)] = nc
    return nc


# ---------------------------------------------------------------- entry point
def kernel(query, reference_points, key_padding_mask, cam_intrinsics,
           cam_extrinsics, feats_l0, feats_l1, feats_l2, feats_l3,
           Wq, bq, Wv, bv, Wo, bo, _trace=False):
    from concourse.bass_utils import run_bass_kernel_spmd

    query = np.asarray(query, np.float32)
    B = query.shape[0]
    idx_all, w_all = _project(np.asarray(reference_points, np.float32),
                              np.asarray(key_padding_mask),
                              np.asarray(cam_intrinsics, np.float32),
                              np.asarray(cam_extrinsics, np.float32))
    patches = _build_patches([np.asarray(f, np.float32)
                              for f in (feats_l0, feats_l1, feats_l2, feats_l3)])
    live = (w_all != 0).any(axis=(2, 3))   # [B, 6, Q]

    # per-core assignment: per batch, pick 2 "half" cams + assign 4 full cams
    # and the 4 half-parts to the 4 cores minimizing the max live-count load.
    import itertools
    plan = [None] * N_CORES
    for b in range(2):
        cnt = [int(live[b, n].sum()) for n in range(6)]
        plo = [int(live[b, n, :QH].sum()) for n in range(6)]
        best = None
        for halves in itertools.combinations(range(6), 2):
            fulls = [n for n in range(6) if n not in halves]
            h1, h2 = halves
            parts = [(h1, 0, plo[h1]), (h1, QH, cnt[h1] - plo[h1]),
                     (h2, 0, plo[h2]), (h2, QH, cnt[h2] - plo[h2])]
            for fp in itertools.permutations(fulls):
                for pp in itertools.permutations(range(4)):
                    load = max(cnt[fp[i]] + parts[pp[i]][2] for i in range(4))
                    if best is None or load < best[0]:
                        best = (load, fp, tuple(parts[pp[i]] for i in range(4)))
        _, fp, pts = best
        for g in range(4):
            n_full = fp[g]
            n_half, qlo, _ = pts[g]
            ql_f = np.where(live[b, n_full])[0]
            ql_h = np.where(live[b, n_half, qlo:qlo + QH])[0]
            if len(ql_f) == 0:
                ql_f = np.array([0])
            if len(ql_h) == 0:
                ql_h = np.array([0])
            plan[4 * b + g] = (b, n_full, n_half, qlo, ql_f, ql_h)
    JF = max(1, -(-max(len(p[4]) for p in plan) // 128))
    JH = max(1, -(-max(len(p[5]) for p in plan) // 128))

    wq4, wv4, wo4 = (_wt4(np.asarray(W, np.float32)) for W in (Wq, Wv, Wo))
    bq2 = np.ascontiguousarray(np.asarray(bq, np.float32).reshape(2, 128).T)
    bv2 = np.ascontiguousarray(np.asarray(bv, np.float32).reshape(2, 128).T)
    bo2 = np.ascontiguousarray(np.asarray(bo, np.float32).reshape(2, 128).T)

    in_maps = []
    for core in range(N_CORES):
        b, n_full, n_half, qlo, ql_f, ql_h = plan[core]
        qT = np.ascontiguousarray(
            query[b].T.reshape(2, 128, QF).transpose(1, 0, 2)).astype(np.float16)
        m_lo, m_hi = (1.0, 0.0) if qlo == 0 else (0.0, 1.0)
        blend_np = np.tile(np.array([m_lo, NEG * (1 - m_lo), m_hi, NEG * (1 - m_hi)],
                                    np.float32), (128, 1))
        in_maps.append({
            "feats_f": patches[b, n_full],
            "feats_h": patches[b, n_half],
            "idx_f": np.stack([_pack_idx(idx_all[b, n_full, l, ql_f], JF * 128, 0)
                               for l in range(4)]),
            "idx_h": np.stack([_pack_idx(idx_all[b, n_half, l, qlo + ql_h], JH * 128, 0)
                               for l in range(4)]),
            "sidx_f": _pack_idx(ql_f, JF * 128, 1000),
            "sidx_h": _pack_idx(ql_h, JH * 128, 500),
            "w_f": _pack_w(w_all[b, n_full][:, :, ql_f], JF),
            "w_h": _pack_w(w_all[b, n_half][:, :, qlo + ql_h], JH),
            "blend": blend_np,
            "queryT": qT,
            "wq": wq4, "wv": wv4, "wo": wo4,
            "bq": bq2, "bv": bv2, "bo": bo2,
        })

    nc = _graph(JF, JH)
    res = run_bass_kernel_spmd(nc, in_maps, core_ids=list(range(N_CORES)),
                               trace=_trace)
    out = np.empty((B, QF, 256), np.float32)
    for b in range(B):
        o = res.results[4 * b]["out"]          # [128, 2, 900]
        out[b] = o.transpose(1, 0, 2).reshape(256, QF).T
    out *= ~np.asarray(key_padding_mask)[..., None]
    if _trace:
        kernel._last_exec_ns = res.exec_time_ns
        it = res.instructions_and_trace
        kernel._last_trace_path = it[1] if it else None
        kernel._last_insts = it[0] if it else None
    return out



# revision 5
# speedup vs baseline: 1.0854x; 1.0854x over previous
"""Multi-camera cross-attention (BEVFormer-style) Trainium2 kernel.

Strategy (8 NeuronCores) — QUERY-sharded, fully independent cores:
  - batch b=0 -> cores 0-3, b=1 -> cores 4-7. The 900 queries of a batch are
    partitioned into 4 groups of 225 (greedy-balanced so that every
    (core, cam) pair has <= 128 live queries -> J=1 gather slabs).
  - Each core samples ALL 6 cameras for its 225 queries, so the max over
    cameras is core-local: no collective, no cross-core barrier.
  - Host precomputes (numpy f32, replicating the reference math exactly):
    projection -> per (cam, level, query) a 2x2-patch base index and 4 slot
    weights (bilinear weights x validity / 4). Dead (cam,query) pairs (~52%)
    are compacted away; live queries are sorted by image position so gather
    descriptors hit nearby DRAM rows.
  - Features are repacked on host to bf16 "patch rows": row (lvl_off+y*W+x) =
    the 4 pixels [(y,x),(y,x+1),(y+1,x),(y+1,x+1)] x 256 ch = 2 KB.
  - Device: per cam ONE dma_gather (all 4 levels, 4*J*128 rows), FMA the
    4 slots x 4 levels with per-partition scalar weights (queries on
    partitions), one dma_scatter_add to un-compact all 6 cams into a zeroed
    DRAM staging buffer, DMA-transpose read back (channels on partitions),
    Wv matmul (PSUM chunks of 512 = 2 cams), max over cams straight out of
    PSUM, then the fused residual + Wq/Wo projections on 225 queries.
"""
import sys
sys.path.insert(0, '/opt/trn_rl_repo')
import numpy as np
import ml_dtypes

BF16 = ml_dtypes.bfloat16
MIN_R, MAX_R = -51.2, 51.2
ORIG_W, ORIG_H = 800.0, 448.0
LEVELS = [(112, 200), (56, 100), (28, 50), (14, 25)]
LVL_OFF = [0, 22400, 28000, 29400]
NPIX = 29750
N_CORES = 8
QC = 225          # queries per core
MS = 1664         # staging rows: 6*256 used + 128 garbage, mult of 128


# ---------------------------------------------------------------- host math
def _project(reference_points, key_padding_mask, cam_intrinsics, cam_extrinsics):
    """idx [B,N,4,Q] int32 (patch base w/ level offset) ; w [B,N,4,4,Q] f32."""
    B, Q, _ = reference_points.shape
    f32 = np.float32
    ref = reference_points.astype(f32) * f32(MAX_R - MIN_R) + f32(MIN_R)
    ref = np.where(key_padding_mask[..., None], f32(-1000.0), ref)
    ref_hom = np.concatenate([ref, np.ones((B, Q, 1), f32)], axis=-1)
    inv_ext = np.linalg.inv(cam_extrinsics.astype(np.float32))
    inv_ext = np.nan_to_num(inv_ext, nan=0.0, posinf=1e6, neginf=-1e6).astype(f32)
    pts_cam_hom = np.einsum('bqj,bnij->bnqi', ref_hom, inv_ext).astype(f32)
    depth = np.nan_to_num(pts_cam_hom[..., 2:3], nan=10.0, posinf=100.0,
                          neginf=-100.0).astype(f32)
    invalid = depth[..., 0] < f32(1.5)
    depth_safe = np.maximum(depth, f32(1.5))
    pts_cam = (pts_cam_hom[..., :3] / depth_safe).astype(f32)
    pts_img = np.einsum('bnqj,bnij->bnqi', pts_cam,
                        cam_intrinsics.astype(f32))[..., :2].astype(f32)
    pts_img = np.clip(pts_img, -3000.0, 3000.0).astype(f32)

    idx_all = np.zeros((B, 6, 4, Q), np.int32)
    w_all = np.zeros((B, 6, 4, 4, Q), np.float32)
    for li, (Hf, Wf) in enumerate(LEVELS):
        fx = pts_img[..., 0] * f32(Wf / ORIG_W)
        fy = pts_img[..., 1] * f32(Hf / ORIG_H)
        gx = np.clip(fx / f32(Wf - 1.0) * f32(2.0) - f32(1.0), -10.0, 10.0).astype(f32)
        gy = np.clip(fy / f32(Hf - 1.0) * f32(2.0) - f32(1.0), -10.0, 10.0).astype(f32)
        gx = np.where(invalid, f32(-100.0), gx)
        gy = np.where(invalid, f32(-100.0), gy)
        px = (gx + f32(1.0)) * f32(0.5) * f32(Wf - 1)
        py = (gy + f32(1.0)) * f32(0.5) * f32(Hf - 1)
        x0 = np.floor(px)
        y0 = np.floor(py)
        x0i = x0.astype(np.int32)
        y0i = y0.astype(np.int32)
        wx1 = px - x0
        wy1 = py - y0
        bx = np.clip(x0i, 0, Wf - 2)
        by = np.clip(y0i, 0, Hf - 2)
        idx_all[:, :, li] = LVL_OFF[li] + by * Wf + bx
        for dx, dy, w in ((0, 0, (1 - wx1) * (1 - wy1)), (1, 0, wx1 * (1 - wy1)),
                          (0, 1, (1 - wx1) * wy1), (1, 1, wx1 * wy1)):
            xi = x0i + dx
            yi = y0i + dy
            valid = (xi >= 0) & (xi < Wf) & (yi >= 0) & (yi < Hf)
            pcx = np.clip(xi, 0, Wf - 1)
            pcy = np.clip(yi, 0, Hf - 1)
            slot = (pcx - bx) + 2 * (pcy - by)
            wv = (w * valid).astype(f32) * f32(0.25)
            for s in range(4):
                w_all[:, :, li, s] += np.where(slot == s, wv, 0.0)
    return idx_all, w_all


def _build_patches(feats_list):
    """4x [B,N,C,H,W] f32 -> [B,N,NPIX,1024] bf16 patch rows."""
    B, N, C = feats_list[0].shape[:3]
    out = np.empty((B, N, NPIX, 4 * C), BF16)
    for li, (Hf, Wf) in enumerate(LEVELS):
        F = feats_list[li].astype(BF16)
        A = np.ascontiguousarray(F.transpose(0, 1, 3, 4, 2))  # [B,N,H,W,C]
        x1 = np.minimum(np.arange(Wf) + 1, Wf - 1)
        y1 = np.minimum(np.arange(Hf) + 1, Hf - 1)
        Ax = A[:, :, :, x1]
        Ay = A[:, :, y1]
        Axy = Ay[:, :, :, x1]
        P = np.stack([A, Ax, Ay, Axy], axis=4)  # [B,N,H,W,4,C]
        out[:, :, LVL_OFF[li]:LVL_OFF[li] + Hf * Wf] = P.reshape(B, N, Hf * Wf, 4 * C)
    return out


def _pack_idx(idx, npad, pad):
    """[k] ints (k<=npad, npad%16==0) -> [128, npad//16] int16 wrapped, x8."""
    buf = np.full((16, npad // 16), pad, np.int16)
    q = len(idx)
    buf[np.arange(q) % 16, np.arange(q) // 16] = np.asarray(idx, np.int16)
    return np.tile(buf, (8, 1))


def _wt4(W):
    """[256,256] weight -> lhsT pack [128, 2, 2, 128] bf16: [p,kb,mb,m]."""
    return np.ascontiguousarray(
        W.T.reshape(2, 128, 2, 128).transpose(1, 0, 2, 3)).astype(BF16)


def _assign_queries(live_b):
    """live_b [6, 900] bool -> list of 4 arrays of 225 query ids, greedy
    balanced so max per-(core,cam) live count is minimized."""
    v = live_b.T.astype(np.int64)          # [900, 6]
    order = np.argsort(-v.sum(axis=1), kind='stable')
    loads = np.zeros((4, 6), np.int64)
    counts = np.zeros(4, np.int64)
    assign = np.empty(900, np.int64)
    for q in order:
        best, bestc = None, None
        for c in range(4):
            if counts[c] >= QC:
                continue
            cost = (np.maximum(loads[c] + v[q], loads).max(),
                    (loads[c] + v[q]).max())
            if best is None or cost < best:
                best, bestc = cost, c
        assign[q] = bestc
        loads[bestc] += v[q]
        counts[bestc] += 1
    return [np.where(assign == c)[0] for c in range(4)], loads.max()


# ---------------------------------------------------------------- device graph
_GRAPHS = {}


def _graph(J):
    if J in _GRAPHS:
        return _GRAPHS[J]
    import concourse.bacc as bacc
    import concourse.mybir as mybir
    from concourse.tile import TileContext
    from concourse.tile_rust import add_dep_helper

    bf16 = mybir.dt.bfloat16
    f32 = mybir.dt.float32
    i16 = mybir.dt.int16
    ALU = mybir.AluOpType
    ACTF = mybir.ActivationFunctionType
    NG = 4 * J * 128          # gathered rows per cam
    NS = 6 * J * 128          # scattered rows (all cams)

    nc = bacc.Bacc(None, num_devices=N_CORES, dynamic_dma_scratch_size=49152)
    feats = [nc.dram_tensor(f"feats_{n}", [NPIX, 1024], bf16,
                            kind="ExternalInput") for n in range(6)]
    idx_d = nc.dram_tensor("idx", [6, 128, NG // 16], i16, kind="ExternalInput")
    sidx_d = nc.dram_tensor("sidx", [128, NS // 16], i16, kind="ExternalInput")
    w_d = nc.dram_tensor("w", [128, 6, 4, 4, J], f32, kind="ExternalInput")
    queryT = nc.dram_tensor("queryT", [128, 2, QC], bf16, kind="ExternalInput")
    wq = nc.dram_tensor("wq", [128, 2, 2, 128], bf16, kind="ExternalInput")
    wv = nc.dram_tensor("wv", [128, 2, 2, 128], bf16, kind="ExternalInput")
    wo = nc.dram_tensor("wo", [128, 2, 2, 128], bf16, kind="ExternalInput")
    bq = nc.dram_tensor("bq", [128, 2], f32, kind="ExternalInput")
    bv = nc.dram_tensor("bv", [128, 2], f32, kind="ExternalInput")
    bo = nc.dram_tensor("bo", [128, 2], f32, kind="ExternalInput")
    out_d = nc.dram_tensor("out", [128, 2, QC], f32, kind="ExternalOutput")
    ms_d = nc.dram_tensor("ms", [MS, 256], bf16, kind="Internal")

    with TileContext(nc) as tc:
        with (
            tc.tile_pool(name="const", bufs=1) as cp,
            tc.tile_pool(name="g", bufs=3) as gp,
            tc.tile_pool(name="wk", bufs=1) as wk,
            tc.tile_pool(name="pv", bufs=8, space="PSUM") as pvp,
        ):
            idx_t = cp.tile([128, 6, NG // 16], i16)
            nc.sync.dma_start(out=idx_t[:], in_=idx_d.rearrange("n p s -> p n s"))
            sidx_t = cp.tile([128, NS // 16], i16)
            nc.sync.dma_start(out=sidx_t[:], in_=sidx_d[:])
            w_t = cp.tile([128, 6, 4, 4, J], f32)
            nc.sync.dma_start(out=w_t[:], in_=w_d[:])
            queryT_t = cp.tile([128, 2, QC], bf16)
            nc.sync.dma_start(out=queryT_t[:], in_=queryT[:])
            wq_t = cp.tile([128, 2, 2, 128], bf16)
            nc.sync.dma_start(out=wq_t[:], in_=wq[:])
            wv_t = cp.tile([128, 2, 2, 128], bf16)
            nc.sync.dma_start(out=wv_t[:], in_=wv[:])
            wo_t = cp.tile([128, 2, 2, 128], bf16)
            nc.sync.dma_start(out=wo_t[:], in_=wo[:])
            bq_t = cp.tile([128, 2], f32)
            nc.sync.dma_start(out=bq_t[:], in_=bq[:])
            bv_t = cp.tile([128, 2], f32)
            nc.sync.dma_start(out=bv_t[:], in_=bv[:])
            bo_t = cp.tile([128, 2], f32)
            nc.sync.dma_start(out=bo_t[:], in_=bo[:])
            zero_t = cp.tile([128, (MS // 128) * 256], bf16)
            nc.vector.memset(zero_t[:], 0.0)
            zi = nc.sync.dma_start(
                out=ms_d.rearrange("(a p) c -> p a c", p=128), in_=zero_t[:])

            # ---- gather + FMA, one gather per cam covering all 4 levels
            acc = wk.tile([128, 6 * J, 256], bf16, name="acc")
            for n in range(6):
                G = gp.tile([128, 4 * J, 1024], bf16, tag="g", name=f"G_{n}")
                nc.gpsimd.dma_gather(G[:], feats[n][:, :], idx_t[:, n, :],
                                     NG, NG, 1024, elem_step=1024)
                for lvl in range(4):
                    for j in range(J):
                        for ps in range(4):
                            in0 = G[:, lvl * J + j, ps * 256:(ps + 1) * 256]
                            sc = w_t[:, n, lvl, ps, j:j + 1]
                            a = acc[:, n * J + j, :]
                            if lvl == 0 and ps == 0:
                                nc.vector.tensor_scalar_mul(a, in0, sc)
                            else:
                                nc.vector.scalar_tensor_tensor(
                                    a, in0, sc, a, ALU.mult, ALU.add)

            # ---- un-compact into staging, transpose back, Wv matmul
            si = nc.gpsimd.dma_scatter_add(ms_d[:, :], acc[:], sidx_t[:],
                                           NS, NS, 256)
            add_dep_helper(si.ins, zi.ins, reason="scatter after zero")
            msT = wk.tile([128, 2, 6 * 256], bf16, name="msT")
            for kb in range(2):
                ti = nc.sync.dma_start(out=msT[:, kb, :],
                                       in_=ms_d[0:1536, kb * 128:(kb + 1) * 128],
                                       transpose=True)
                add_dep_helper(ti.ins, si.ins, reason="tread after scatter")

            pts = {}
            for mb in range(2):
                for ch in range(3):          # 512 cols = 2 cams per chunk
                    pt = pvp.tile([128, 512], f32, tag="pv",
                                  name=f"pv_{mb}_{ch}")
                    for kb in range(2):
                        nc.tensor.matmul(pt[:], lhsT=wv_t[:, kb, mb, :],
                                         rhs=msT[:, kb, ch * 512:(ch + 1) * 512],
                                         start=(kb == 0), stop=(kb == 1))
                    pts[(mb, ch)] = pt

            # ---- max over 6 cams (straight out of PSUM) + fused tail
            s = wk.tile([128, 2, QC], f32, name="s")
            for mb in range(2):
                tt = []
                for ch in range(3):
                    c1 = wk.tile([128, QC], f32, tag=f"c1_{mb}_{ch}")
                    nc.scalar.activation(c1[:], pts[(mb, ch)][:, 256:256 + QC],
                                         ACTF.Copy)
                    tm = wk.tile([128, QC], f32, tag=f"tm_{mb}_{ch}")
                    nc.vector.tensor_tensor(out=tm[:],
                                            in0=pts[(mb, ch)][:, 0:QC],
                                            in1=c1[:], op=ALU.max)
                    tt.append(tm)
                nc.vector.tensor_tensor(out=tt[0][:], in0=tt[0][:],
                                        in1=tt[1][:], op=ALU.max)
                nc.vector.tensor_tensor(out=s[:, mb, :], in0=tt[0][:],
                                        in1=tt[2][:], op=ALU.max)

            qT = wk.tile([128, 2, QC], f32, name="qT")
            for mb in range(2):
                pq = pvp.tile([128, QC], f32, tag="pv", name=f"pq_{mb}")
                for kb in range(2):
                    nc.tensor.matmul(pq[:], lhsT=wq_t[:, kb, mb, :],
                                     rhs=queryT_t[:, kb, :],
                                     start=(kb == 0), stop=(kb == 1))
                nc.vector.tensor_scalar_add(qT[:, mb, :], pq[:],
                                            bq_t[:, mb:mb + 1])
            t1 = wk.tile([128, 2, QC], f32, name="t1")
            for mb in range(2):
                nc.vector.scalar_tensor_tensor(t1[:, mb, :], s[:, mb, :],
                                               bv_t[:, mb:mb + 1], qT[:, mb, :],
                                               ALU.add, ALU.add)
            t2 = wk.tile([128, 2, QC], f32, name="t2")
            nc.scalar.activation(t2[:], t1[:], ACTF.Relu)
            fusedT = wk.tile([128, 2, QC], bf16, name="fusedT")
            nc.vector.tensor_tensor(out=fusedT[:], in0=t2[:], in1=qT[:],
                                    op=ALU.add)
            outT = wk.tile([128, 2, QC], f32, name="outT")
            for mb in range(2):
                po = pvp.tile([128, QC], f32, tag="pv", name=f"po_{mb}")
                for kb in range(2):
                    nc.tensor.matmul(po[:], lhsT=wo_t[:, kb, mb, :],
                                     rhs=fusedT[:, kb, :],
                                     start=(kb == 0), stop=(kb == 1))
                nc.vector.tensor_scalar_add(outT[:, mb, :], po[:],
                                            bo_t[:, mb:mb + 1])
            nc.sync.dma_start(out=out_d[:], in_=outT[:])
    nc.compile()
    _GRAPHS[J] = nc
    return nc


# ---------------------------------------------------------------- entry point
def kernel(query, reference_points, key_padding_mask, cam_intrinsics,
           cam_extrinsics, feats_l0, feats_l1, feats_l2, feats_l3,
           Wq, bq, Wv, bv, Wo, bo, _trace=False):
    from concourse.bass_utils import run_bass_kernel_spmd

    query = np.asarray(query, np.float32)
    mask = np.asarray(key_padding_mask)
    B = query.shape[0]
    idx_all, w_all = _project(np.asarray(reference_points, np.float32), mask,
                              np.asarray(cam_intrinsics, np.float32),
                              np.asarray(cam_extrinsics, np.float32))
    patches = _build_patches([np.asarray(f, np.float32)
                              for f in (feats_l0, feats_l1, feats_l2, feats_l3)])
    live = (w_all != 0).any(axis=(2, 3))   # [B, 6, Q]

    plan = []                              # (b, qsel, [ql per cam])
    maxload = 0
    for b in range(B):
        qsels, ml = _assign_queries(live[b])
        maxload = max(maxload, ml)
        for qsel in qsels:
            qls = []
            for n in range(6):
                ql = np.where(live[b, n, qsel])[0]   # local ids into qsel
                # sort by image position (level-0 patch index) for locality
                ql = ql[np.argsort(idx_all[b, n, 0, qsel[ql]], kind='stable')]
                qls.append(ql)
            plan.append((b, qsel, qls))
    J = max(1, -(-int(maxload) // 128))

    wq4, wv4, wo4 = (_wt4(np.asarray(W, np.float32)) for W in (Wq, Wv, Wo))
    bq2 = np.ascontiguousarray(np.asarray(bq, np.float32).reshape(2, 128).T)
    bv2 = np.ascontiguousarray(np.asarray(bv, np.float32).reshape(2, 128).T)
    bo2 = np.ascontiguousarray(np.asarray(bo, np.float32).reshape(2, 128).T)

    NG, NS = 4 * J * 128, 6 * J * 128
    in_maps = []
    for core in range(N_CORES):
        b, qsel, qls = plan[core]
        qT = np.ascontiguousarray(
            query[b][qsel].T.reshape(2, 128, QC).transpose(1, 0, 2)).astype(BF16)
        idx_pack = np.empty((6, 128, NG // 16), np.int16)
        w_pack = np.zeros((128, 6, 4, 4, J), np.float32)
        sidx = np.full(NS, MS - 128, np.int64)   # pads -> garbage row
        for n in range(6):
            ql = qls[n]
            gq = qsel[ql]
            rows = np.concatenate(
                [np.pad(idx_all[b, n, l, gq], (0, J * 128 - len(ql)))
                 for l in range(4)])
            idx_pack[n] = _pack_idx(rows, NG, 0)
            k = np.arange(len(ql))
            w_pack[k % 128, n, :, :, k // 128] = \
                w_all[b, n][:, :, gq].transpose(2, 0, 1)
            sidx[n * J * 128 + k] = n * 256 + ql
        in_map = {
            "idx": idx_pack,
            "sidx": _pack_idx(sidx, NS, MS - 128),
            "w": w_pack,
            "queryT": qT,
            "wq": wq4, "wv": wv4, "wo": wo4,
            "bq": bq2, "bv": bv2, "bo": bo2,
        }
        for n in range(6):
            in_map[f"feats_{n}"] = patches[b, n]
        in_maps.append(in_map)

    nc = _graph(J)
    res = run_bass_kernel_spmd(nc, in_maps, core_ids=list(range(N_CORES)),
                               trace=_trace)
    out = np.empty((B, 900, 256), np.float32)
    for core in range(N_CORES):
        b, qsel, _ = plan[core]
        o = res.results[core]["out"]          # [128, 2, QC]
        out[b, qsel] = o.transpose(1, 0, 2).reshape(256, QC).T
    out *= ~mask[..., None]
    if _trace:
        kernel._last_exec_ns = res.exec_time_ns
        it = res.instructions_and_trace
        kernel._last_trace_path = it[1] if it else None
    return out
